# revision 19
# baseline (speedup 1.0000x reference)
"""VDP (variance-propagating) attention kernel for Trainium2, 8 NeuronCores.

Sharding: core c -> (batch b = c//2, head-group g = c%2) [8 heads each].
Each core computes LN + its QKV slice + attention for its 8 heads + the
partial out-projection for its 512 inner columns. Host sums the two
head-group partials per batch. No collectives needed.

Layout trick: everything on-device lives transposed as [feature, token]
(activations) / [contraction, out] (weights), prepared host-side, so the
contraction dim is always on partitions and no on-device transposes are
needed anywhere. LayerNorm stats (reduce over features = partitions) are
done with ones-vector matmuls on the PE; softmax denominators come for
free from a ones-augmented column in the V operand of the mu-attention AV
matmul, and are broadcast back across partitions with a K=1 PE matmul.

Perf notes (CoreSim cost model): matmul cost = out_free x cyc(moving
dtype): fp32 4.0, fp32r 1.0 (free>=256), bf16 1.0, fp8e4+DoubleRow 0.5
with twice the contraction per instruction. The whole sigma path
(positive-sum accumulations) runs as fp8 DoubleRow pairs; mu path stays
bf16; LN stats run on raw fp32 data viewed as fp32r. All activations use
one table (Exp/Ln/Square/Copy) - sqrt is exp(-0.5*ln(var+eps)).

Host pre-scaling: mu-weights x sqrt(512) (un-done at PSUM eviction);
sigma-raw weights get +ln(512) so a single Exp yields 512*softplus(x)
(relative error ~ softplus(x)/2 ~ 0.3% at x ~ -5).
"""

import math
import os
import sys

import numpy as np

for _p in ("/opt/trn_rl_repo", "/root/.axon_site/_ro/trn_rl_repo"):
    if os.path.isdir(_p) and _p not in sys.path:
        sys.path.insert(0, _p)

HEADS = 16
DH = 64
SCALE = DH ** -0.5
EPS = 1e-5
B, N, D = 4, 1024, 1024
HPC = 8          # heads per core
RQK = 1024       # q+k rows per core (2 * 8 heads * 64)
RV = 512         # v rows per core
P = 128

SQ512 = math.sqrt(512.0)          # host scale on mu-weights
LN512 = math.log(512.0)           # host shift on sigma-raw weights
WB = 512.0                        # w fp8 boost (w_true max ~0.14 -> ~70)
SQWB = math.sqrt(WB)
IS512 = 1.0 / 512.0
ISQ512 = 1.0 / SQ512
ISWB = 1.0 / WB
CBIAS = 10.0    # fp8 mean-shift for oT_sg/a2o (compensated via colsum matmul)

_NC_CACHE = {}


def _build_nc(tiny_out=False):
    import concourse.bass as bass  # noqa: F401
    import concourse.tile as tile
    from concourse import bacc, mybir

    f32 = mybir.dt.float32
    AF = mybir.ActivationFunctionType
    ALU = mybir.AluOpType

    nc = bacc.Bacc(None, target_bir_lowering=False)

    io = {}
    for name, shape in [
        ("muT", [D, N]), ("sgT", [D, N]), ("gb", [P, 16]),
        ("wqk_mu", [D, RQK]), ("wqk_sr", [D, RQK]),
        ("wv_mu", [D, RV]), ("wv_sr", [D, RV]),
        ("wo_mu", [RV, D]), ("wo_sr", [RV, D]), ("onesd", [P, P]),
    ]:
        io[name] = nc.dram_tensor(name, shape, f32, kind="ExternalInput")
    if tiny_out:
        for name, shape in [("yT_mu", [D, N]), ("yT_sg", [D, N])]:
            io[name] = nc.dram_tensor(name, shape, f32)
        io["done"] = nc.dram_tensor("done", [1, 16], f32, kind="ExternalOutput")
    else:
        for name, shape in [("yT_mu", [D, N]), ("yT_sg", [D, N])]:
            io[name] = nc.dram_tensor(name, shape, f32, kind="ExternalOutput")

    with tile.TileContext(nc) as tc:
        _emit(nc, tc, io, mybir)
        if tiny_out:
            with tc.tile_pool(name="doneP", bufs=1) as dp:
                dt_t = dp.tile([1, 16], f32)
                nc.vector.memset(dt_t, 1.0)
                nc.sync.dma_start(out=io["done"][:], in_=dt_t)
    nc.compile()
    return nc


def _emit(nc, tc, io, mybir):
    from contextlib import ExitStack

    f32 = mybir.dt.float32
    f32r = mybir.dt.float32r
    bf = mybir.dt.bfloat16
    f8 = mybir.dt.float8e4
    AF = mybir.ActivationFunctionType
    ALU = mybir.AluOpType
    DR = mybir.MatmulPerfMode.DoubleRow

    with ExitStack() as tctx:
        stage = tctx.enter_context(tc.tile_pool(name="stage", bufs=1))
        # persistent SBUF staging: no DRAM round trips between phases
        qk_mu_sb = stage.tile([P, 8, N], bf)    # rows: 0-3 q-blocks, 4-7 k-blocks
        qk_sg_sb = stage.tile([P, 8, N], bf)    # scaled x SQWB (q also x SCALE)
        v_mu_sb = stage.tile([P, 8, HPC * 65], bf)   # per tok-blk: 8 heads x (64 v + ones)
        v_sg_sb = stage.tile([P, 8, RV], bf)
        oT_mu_sb = stage.tile([P, 4, N], f32r)
        co_dr = stage.tile([P, 4, 2, N], f8)    # plane0 = a2o, plane1 = oT_sg
        mu_nT = stage.tile([P, 8, N], f32r)
        as_dr = stage.tile([P, 8, 2, N], f8)    # plane0 = a2T, plane1 = sg_nT
        ones3 = stage.tile([P, P], f32r)
        nc.gpsimd.dma_start(out=ones3, in_=io["onesd"][:])

        # ============ Phase A: LayerNorm + QKV ============
        with ExitStack() as actx:
            acts = actx.enter_context(tc.tile_pool(name="acts", bufs=1))
            smallA = actx.enter_context(tc.tile_pool(name="smallA", bufs=1))

            gb_sb = smallA.tile([P, 16], f32)
            nc.sync.dma_start(out=gb_sb, in_=io["gb"][:])
            g2_sb = smallA.tile([P, 8], f32)
            nc.vector.tensor_mul(g2_sb, gb_sb[:, 0:8], gb_sb[:, 0:8])

            eps1 = smallA.tile([1, 1], f32)
            nc.vector.memset(eps1, EPS)

            inv_b = acts.tile([P, N], f32)
            minv_b = acts.tile([P, N], f32)
            inv2_b = acts.tile([P, N], f32)

            # --- A1: stats + normalize (muT streamed twice, not resident) ---
            with ExitStack() as ctx:
                ioA = ctx.enter_context(tc.tile_pool(name="ioA", bufs=2))
                psS = ctx.enter_context(tc.tile_pool(name="psS", bufs=1, space="PSUM"))
                psA = ctx.enter_context(tc.tile_pool(name="psA", bufs=2, space="PSUM"))

                sum_ps = [psS.tile([1, 512], f32, tag=f"sum{c}", name=f"sum{c}") for c in range(2)]
                sq_ps = [psS.tile([1, 512], f32, tag=f"sq{c}", name=f"sq{c}") for c in range(2)]
                for j in range(8):
                    mut = ioA.tile([P, N], f32r, tag="mut")
                    nc.gpsimd.dma_start(out=mut, in_=io["muT"][j * P:(j + 1) * P, :])
                    mu2 = ioA.tile([P, N], f32r, tag="mu2")
                    nc.gpsimd.tensor_mul(mu2, mut, mut)
                    for c in range(2):
                        cs = slice(c * 512, (c + 1) * 512)
                        nc.tensor.matmul(sum_ps[c], ones3[:, 0:1], mut[:, cs],
                                         start=(j == 0), stop=(j == 7), skip_group_check=True)
                        nc.tensor.matmul(sq_ps[c], ones3[:, 0:1], mu2[:, cs],
                                         start=(j == 0), stop=(j == 7), skip_group_check=True)

                inv_sb = smallA.tile([1, N], f32r)
                minv_sb = smallA.tile([1, N], f32r)
                for c in range(2):
                    cs = slice(c * 512, (c + 1) * 512)
                    mean_t = ioA.tile([1, 512], f32, tag="mean")
                    nc.vector.tensor_scalar_mul(mean_t, sum_ps[c], 1.0 / D)
                    m2_t = ioA.tile([1, 512], f32, tag="m2")
                    nc.vector.tensor_mul(m2_t, mean_t, mean_t)
                    var_t = ioA.tile([1, 512], f32, tag="var")
                    nc.vector.scalar_tensor_tensor(var_t, sq_ps[c], 1.0 / D, m2_t,
                                                   ALU.mult, ALU.subtract)
                    # inv = exp(-0.5 * ln(var + eps)): stays on the one act table
                    lv_t = ioA.tile([1, 512], f32, tag="lv")
                    nc.scalar.activation(lv_t, var_t, AF.Ln, bias=eps1)
                    nc.scalar.activation(inv_sb[:, cs], lv_t, AF.Exp, scale=-0.5)
                    nc.vector.scalar_tensor_tensor(minv_sb[:, cs], mean_t, -1.0, inv_sb[:, cs],
                                                   ALU.mult, ALU.mult)

                for c in range(2):
                    cs = slice(c * 512, (c + 1) * 512)
                    bp1 = psA.tile([P, 512], f32, tag="bcast")
                    nc.tensor.matmul(bp1, ones3[0:1, :], inv_sb[:, cs], start=True, stop=True)
                    nc.scalar.copy(inv_b[:, cs], bp1)
                    bp2 = psA.tile([P, 512], f32, tag="bcast")
                    nc.tensor.matmul(bp2, ones3[0:1, :], minv_sb[:, cs], start=True, stop=True)
                    nc.vector.tensor_copy(minv_b[:, cs], bp2)
                nc.vector.tensor_mul(inv2_b, inv_b, inv_b)

                for j in range(8):
                    mut = ioA.tile([P, N], f32, tag="mut")
                    nc.sync.dma_start(out=mut, in_=io["muT"][j * P:(j + 1) * P, :])
                    x2 = ioA.tile([P, N], f32, tag="x2")
                    nc.vector.scalar_tensor_tensor(x2, mut, gb_sb[:, j:j + 1], inv_b,
                                                   ALU.mult, ALU.mult)
                    cb = ioA.tile([P, N], f32, tag="cb")
                    nc.vector.tensor_scalar(cb, minv_b, gb_sb[:, j:j + 1],
                                            gb_sb[:, 8 + j:9 + j], ALU.mult, ALU.add)
                    nc.gpsimd.tensor_add(mu_nT[:, j, :], x2, cb)
                    sgt = ioA.tile([P, N], f32, tag="sgt")
                    nc.sync.dma_start(out=sgt, in_=io["sgT"][j * P:(j + 1) * P, :])
                    nc.vector.scalar_tensor_tensor(as_dr[:, j, 1, :], sgt, g2_sb[:, j:j + 1],
                                                   inv2_b, ALU.mult, ALU.mult)
                    z = ioA.tile([P, N], bf, tag="z")
                    nc.scalar.activation(z, mu_nT[:, j, :], AF.Square)
                    nc.gpsimd.tensor_add(as_dr[:, j, 0, :], as_dr[:, j, 1, :], z)

            # --- A2a: QKV q,k rows (transposed out), evict straight to SBUF stage ---
            with ExitStack() as ctx:
                wq = ctx.enter_context(tc.tile_pool(name="wq", bufs=2))
                psQ = ctx.enter_context(tc.tile_pool(name="psQ", bufs=2, space="PSUM"))
                for rb in range(8):
                    rsl = slice(rb * P, (rb + 1) * P)
                    wmu = wq.tile([P, 8, P], f32r, tag="wmu")
                    nc.gpsimd.dma_start(out=wmu, in_=io["wqk_mu"][:, rsl].rearrange("(j p) r -> p j r", p=P))
                    wsr = wq.tile([P, 8, P], f32, tag="wsr")
                    nc.sync.dma_start(out=wsr, in_=io["wqk_sr"][:, rsl].rearrange("(j p) r -> p j r", p=P))
                    wq_dr = wq.tile([P, 8, 2, P], f8, tag="wq_dr")
                    nc.scalar.activation(wq_dr[:, :, 0, :], wsr, AF.Exp)
                    nc.gpsimd.tensor_mul(wq_dr[:, :, 1, :], wmu, wmu)
                    for c in range(2):
                        cs = slice(c * 512, (c + 1) * 512)
                        ps_mu = psQ.tile([P, 512], f32, tag="qkmu")
                        for j in range(8):
                            nc.tensor.matmul(ps_mu, wmu[:, j, :], mu_nT[:, j, cs],
                                             start=(j == 0), stop=(j == 7))
                        nc.vector.tensor_scalar_mul(qk_mu_sb[:, rb, cs], ps_mu, ISQ512)
                        ps_sg = psQ.tile([P, 512], f32, tag="qksg")
                        for j in range(8):
                            nc.tensor.matmul(ps_sg, wq_dr[:, j, :, :], as_dr[:, j, :, cs],
                                             start=(j == 0), stop=(j == 7), perf_mode=DR)
                        sgev = (SCALE if rb < 4 else 1.0) * SQWB * IS512
                        nc.scalar.activation(qk_sg_sb[:, rb, cs], ps_sg, AF.Copy, scale=sgev)

            # --- A2b: V (natural layout), evict straight to SBUF stage ---
            with ExitStack() as ctx:
                wv = ctx.enter_context(tc.tile_pool(name="wv", bufs=1))
                psV = ctx.enter_context(tc.tile_pool(name="psV", bufs=2, space="PSUM"))
                wv_mu = wv.tile([P, 8, 512], f32r)
                nc.gpsimd.dma_start(out=wv_mu, in_=io["wv_mu"][:].rearrange("(j p) r -> p j r", p=P))
                wv_sr = wv.tile([P, 8, 512], f32)
                nc.sync.dma_start(out=wv_sr, in_=io["wv_sr"][:].rearrange("(j p) r -> p j r", p=P))
                wv_dr = wv.tile([P, 8, 2, 512], f8)
                nc.scalar.activation(wv_dr[:, :, 0, :], wv_sr, AF.Exp)
                nc.gpsimd.tensor_mul(wv_dr[:, :, 1, :], wv_mu, wv_mu)
                nc.vector.memset(v_mu_sb, 1.0)
                for tb in range(8):
                    tsl = slice(tb * P, (tb + 1) * P)
                    ps_mu = psV.tile([P, 512], f32, tag="vmu")
                    for j in range(8):
                        nc.tensor.matmul(ps_mu, mu_nT[:, j, tsl], wv_mu[:, j, :],
                                         start=(j == 0), stop=(j == 7))
                    nc.vector.tensor_scalar_mul(
                        v_mu_sb[:, tb, :].rearrange("p (h c) -> p h c", c=65)[:, :, 0:64],
                        ps_mu.rearrange("p (h c) -> p h c", c=64), ISQ512)
                    ps_sg = psV.tile([P, 512], f32, tag="vsg")
                    for j in range(8):
                        nc.tensor.matmul(ps_sg, as_dr[:, j, :, tsl], wv_dr[:, j, :, :],
                                         start=(j == 0), stop=(j == 7), perf_mode=DR)
                    nc.scalar.activation(v_sg_sb[:, tb, :], ps_sg,
                                         AF.Copy, scale=IS512)

        # ============ Phase B: attention (all operands already in SBUF) ============
        with ExitStack() as ctx:
            ep = ctx.enter_context(tc.tile_pool(name="ep", bufs=18))
            sb3 = ctx.enter_context(tc.tile_pool(name="sb3", bufs=4))
            ptu = ctx.enter_context(tc.tile_pool(name="ptu", bufs=2))
            outsb = ctx.enter_context(tc.tile_pool(name="outsb", bufs=4))
            smallB = ctx.enter_context(tc.tile_pool(name="smallB", bufs=4))
            wpool = ctx.enter_context(tc.tile_pool(name="wpool", bufs=3))
            psD = ctx.enter_context(tc.tile_pool(name="psD", bufs=1, space="PSUM"))
            psS2 = ctx.enter_context(tc.tile_pool(name="psS2", bufs=1, space="PSUM"))
            psAVm = ctx.enter_context(tc.tile_pool(name="psAVm", bufs=2, space="PSUM"))
            psAVs = ctx.enter_context(tc.tile_pool(name="psAVs", bufs=1, space="PSUM"))
            psDB = ctx.enter_context(tc.tile_pool(name="psDB", bufs=1, space="PSUM"))



            def pass1(hq, c):
                pr, hh = divmod(hq, 2)
                pb = (hq % 2) * 64
                qrb, krb = hq // 2, 4 + hq // 2
                vco = pr * 130 + hh * 65
                cs = slice(c * 512, (c + 1) * 512)
                av_mu = psAVm.tile([65, 512], f32, tag="avmu", name=f"avmu{hq}_{c}")
                e_ts = []
                for kp in range(4):
                    e2 = ep.tile([P, 2, 512], bf, tag="e", name=f"e{hq}_{c}_{kp}")
                    e_ts.append(e2)
                    dots2 = psD.tile([P, 2, 512], f32, tag="dots", name=f"dots{hq}_{c}_{kp}")
                    for i in range(2):
                        kb = 2 * kp + i
                        nc.tensor.matmul(dots2[:, i, :],
                                         qk_mu_sb[pb:pb + 64, krb, kb * P:(kb + 1) * P],
                                         qk_mu_sb[pb:pb + 64, qrb, cs],
                                         start=True, stop=True, skip_group_check=True)
                    nc.scalar.activation(e2, dots2, AF.Exp, scale=SCALE)
                    for i in range(2):
                        kb = 2 * kp + i
                        nc.tensor.matmul(av_mu, v_mu_sb[:, kb, vco:vco + 65], e2[:, i, :],
                                         start=(kb == 0), stop=(kb == 7))
                r_sb = smallB.tile([P, 512], f32r, tag="r", name=f"r{hq}_{c}")
                with nc.allow_low_precision(reason="f32r keeps full fp32 bytes here"):
                    nc.vector.reciprocal(r_sb[64:65, :], av_mu[64:65, :])
                dbp = psDB.tile([P, 512], f32, tag="db", name=f"dbp{hq}_{c}")
                nc.tensor.matmul(dbp, ones3[64:65, :], r_sb[64:65, :], start=True, stop=True)
                db = sb3.tile([P, 512], bf, tag="db_sb", name=f"db{hq}_{c}")
                nc.scalar.copy(db, dbp)
                muo = outsb.tile([64, 512], f32r, tag="muo", name=f"muo{hq}_{c}")
                nc.vector.tensor_mul(muo, av_mu[0:64, :], db[0:64, :])
                nc.sync.dma_start(out=oT_mu_sb[pb:pb + 64, qrb, cs], in_=muo)
                return (hq, c, e_ts, db)

            def pass2(stateA, stateB):
                # both heads of a pair: sigma-AV DoubleRow matmuls col-packed
                # via tile_position (0,0)/(0,64) and kb-paired fp8 planes.
                hqA, c, e_tsA, dbA = stateA
                hqB, _, e_tsB, dbB = stateB
                pr = hqA // 2
                qrb, krb = pr, 4 + pr
                cs = slice(c * 512, (c + 1) * 512)
                av2 = psAVs.tile([P, 512], f32, tag="avsg", name=f"avsg{hqA}_{c}")
                for kp in range(4):
                    for hq, pb, e_ts, db in ((hqA, 0, e_tsA, dbA), (hqB, 64, e_tsB, dbB)):
                        sd2 = psS2.tile([P, 2, 512], f32, tag="sd2", name=f"sd{hq}_{c}_{kp}")
                        for i in range(2):
                            kb = 2 * kp + i
                            nc.tensor.matmul(sd2[:, i, :],
                                             qk_sg_sb[pb:pb + 64, krb, kb * P:(kb + 1) * P],
                                             qk_sg_sb[pb:pb + 64, qrb, cs],
                                             start=True, stop=True, skip_group_check=True)
                        sde = ptu.tile([P, 2, 512], bf, tag="sde", name=f"sde{hq}_{c}_{kp}")
                        nc.scalar.copy(sde, sd2)
                        p2 = ptu.tile([P, 2, 512], bf, tag="p", name=f"p{hq}_{c}_{kp}")
                        nc.vector.tensor_mul(
                            p2, e_ts[kp],
                            db[:, :].rearrange("p (o f) -> p o f", o=1)
                            .to_broadcast((P, 2, 512)))
                        q2 = ptu.tile([P, 2, 512], bf, tag="q", name=f"q{hq}_{c}_{kp}")
                        nc.gpsimd.tensor_mul(q2, p2, p2)
                        t2 = ptu.tile([P, 2, 512], bf, tag="t", name=f"t{hq}_{c}_{kp}")
                        nc.gpsimd.tensor_sub(t2, p2, q2)
                        u2 = ptu.tile([P, 2, 512], bf, tag="u", name=f"u{hq}_{c}_{kp}")
                        nc.vector.tensor_mul(u2, t2, t2)
                        w2 = wpool.tile([P, 2, 512], bf, tag="w", name=f"w{hq}_{c}_{kp}")
                        nc.vector.tensor_mul(w2, u2, sde)
                        for i in range(2):
                            kb = 2 * kp + i
                            nc.tensor.matmul(av2[pb:pb + 64, :],
                                             v_sg_sb[:, kb, hq * 64:(hq + 1) * 64],
                                             w2[:, i, :],
                                             start=(kb == 0), stop=(kb == 7),
                                             tile_position=(0, pb),
                                             skip_group_check=True)
                nc.scalar.activation(co_dr[:, qrb, 1, cs], av2, AF.Copy,
                                     scale=ISWB, bias=-CBIAS)

            prev = None
            for pr in range(4):
                for c in range(2):
                    curA = pass1(2 * pr, c)
                    curB = pass1(2 * pr + 1, c)
                    if prev is not None:
                        pass2(*prev)
                    prev = (curA, curB)
            pass2(*prev)

        # ============ Phase C: out-projection ============
        with ExitStack() as ctx:
            wo = ctx.enter_context(tc.tile_pool(name="wo", bufs=1))
            oin = ctx.enter_context(tc.tile_pool(name="oin", bufs=1))
            evC = ctx.enter_context(tc.tile_pool(name="evC", bufs=4))
            psC = ctx.enter_context(tc.tile_pool(name="psC", bufs=2, space="PSUM"))

            wo_mu = wo.tile([P, 4, D], f32r)
            nc.gpsimd.dma_start(out=wo_mu, in_=io["wo_mu"][:].rearrange("(j p) o -> p j o", p=P))
            wo_sr = wo.tile([P, 4, D], f32)
            nc.sync.dma_start(out=wo_sr, in_=io["wo_sr"][:].rearrange("(j p) o -> p j o", p=P))
            wo_dr = wo.tile([P, 4, 2, D], f8)
            nc.scalar.activation(wo_dr[:, :, 0, :], wo_sr, AF.Exp)
            nc.gpsimd.tensor_mul(wo_dr[:, :, 1, :], wo_mu, wo_mu)
            ones_c = wo.tile([P, 2, 1], f8)
            nc.vector.memset(ones_c, 1.0)

            for j in range(4):
                zsq = oin.tile([P, N], bf, tag="zsq", name=f"zsq{j}")
                nc.scalar.activation(zsq, oT_mu_sb[:, j, :], AF.Square)
                nc.gpsimd.tensor_add(co_dr[:, j, 0, :], co_dr[:, j, 1, :], zsq)

            for ob in range(8):
                osl = slice(ob * P, (ob + 1) * P)
                colsum = psC.tile([P, 1], f32, tag="colsum", name=f"cols{ob}")
                for j in range(4):
                    nc.tensor.matmul(colsum, wo_dr[:, j, :, osl], ones_c,
                                     start=(j == 0), stop=(j == 3), perf_mode=DR)
                bias_col = evC.tile([P, 1], f32, tag="bias_col", name=f"bcol{ob}")
                nc.vector.tensor_scalar_mul(bias_col, colsum, CBIAS * IS512)
                for c in range(2):
                    cs = slice(c * 512, (c + 1) * 512)
                    ps_mu = psC.tile([P, 512], f32, tag="ymu")
                    for j in range(4):
                        nc.tensor.matmul(ps_mu, wo_mu[:, j, osl], oT_mu_sb[:, j, cs],
                                         start=(j == 0), stop=(j == 3))
                    ev1 = evC.tile([P, 512], f32, tag="ev1")
                    nc.vector.tensor_scalar_mul(ev1, ps_mu, ISQ512)
                    nc.sync.dma_start(out=io["yT_mu"][osl, cs], in_=ev1)
                    ps_sg = psC.tile([P, 512], f32, tag="ysg")
                    for j in range(4):
                        nc.tensor.matmul(ps_sg, wo_dr[:, j, :, osl], co_dr[:, j, :, cs],
                                         start=(j == 0), stop=(j == 3), perf_mode=DR)
                    ev2 = evC.tile([P, 512], f32, tag="ev2")
                    nc.scalar.activation(ev2, ps_sg, AF.Identity, scale=IS512,
                                         bias=bias_col)
                    nc.sync.dma_start(out=io["yT_sg"][osl, cs], in_=ev2)


def _get_nc():
    if "nc" not in _NC_CACHE:
        _NC_CACHE["nc"] = _build_nc()
    return _NC_CACHE["nc"]


def _sp512(x):
    # device computes Exp(this) -> exactly 512*softplus(x)
    x64 = np.asarray(x, np.float64)
    return np.asarray(np.log(512.0 * np.log1p(np.exp(x64))), np.float32)


def _prep_core_inputs(c, mu, sigma, ln_gamma, ln_beta, Wqkv_mu, Wqkv_sigma_raw,
                      Wout_mu, Wout_sigma_raw):
    f = np.float32
    asc = np.ascontiguousarray
    b, g = divmod(c, 2)
    qs = slice(512 * g, 512 * (g + 1))
    ks = slice(1024 + 512 * g, 1024 + 512 * (g + 1))
    vs = slice(2048 + 512 * g, 2048 + 512 * (g + 1))
    gb = np.zeros((P, 16), f)
    gb[:, :8] = np.asarray(ln_gamma, f).reshape(8, P).T
    gb[:, 8:] = np.asarray(ln_beta, f).reshape(8, P).T
    wqk_mu = np.concatenate([Wqkv_mu[qs], Wqkv_mu[ks]], 0)
    wqk_sr = np.concatenate([Wqkv_sigma_raw[qs], Wqkv_sigma_raw[ks]], 0)
    return {
        "muT": asc(np.asarray(mu[b], f).T),
        "sgT": asc(np.asarray(sigma[b], f).T),
        "gb": gb,
        "wqk_mu": asc(np.asarray(wqk_mu, f).T) * f(SQ512),
        "wqk_sr": _sp512(asc(np.asarray(wqk_sr, f).T)),
        "wv_mu": asc(np.asarray(Wqkv_mu[vs], f).T) * f(SQ512),
        "wv_sr": _sp512(asc(np.asarray(Wqkv_sigma_raw[vs], f).T)),
        "wo_mu": asc(np.asarray(Wout_mu[:, 512 * g:512 * (g + 1)], f).T) * f(SQ512),
        "wo_sr": _sp512(asc(np.asarray(Wout_sigma_raw[:, 512 * g:512 * (g + 1)], f).T)),
        "onesd": np.ones((P, P), f),
    }


def _emulate_core(m):
    """Pure-numpy mirror of the on-device program (for validation only)."""
    import ml_dtypes

    def q8(x):
        return np.asarray(x, np.float32).astype(ml_dtypes.float8_e4m3).astype(np.float32)

    def qb(x):
        return np.asarray(x, np.float32).astype(ml_dtypes.bfloat16).astype(np.float32)

    muT, sgT = m["muT"], m["sgT"]
    gamma = m["gb"][:, :8].T.reshape(-1)[:, None]   # [D,1] indexed by d
    beta = m["gb"][:, 8:].T.reshape(-1)[:, None]
    mean = muT.mean(0, keepdims=True)
    var = muT.var(0, keepdims=True)
    inv = np.exp(-0.5 * np.log(var + EPS))
    mu_nT = qb((muT * inv - mean * inv) * gamma + beta)
    sg_nT = q8(sgT * gamma * gamma * inv * inv)
    a2T = q8(mu_nT * mu_nT + sg_nT)
    wsig_qk = q8(np.exp(m["wqk_sr"]))        # = 512*softplus exact (host-prescaled)
    wmu2_qk = q8(m["wqk_mu"] ** 2)           # = 512*Wmu^2 (prescaled)
    qkT_mu = qb((m["wqk_mu"].T @ mu_nT) * ISQ512)
    qkT_sg = qb((wsig_qk.T @ a2T + wmu2_qk.T @ sg_nT) * IS512 * SQWB)
    qkT_sg[:512] *= SCALE
    wsig_v = q8(np.exp(m["wv_sr"]))
    wmu2_v = q8(m["wv_mu"] ** 2)
    v_mu = qb((mu_nT.T @ m["wv_mu"]) * ISQ512)
    v_sg = qb((a2T.T @ wsig_v + sg_nT.T @ wmu2_v) * IS512)
    oT_mu = np.zeros((RV, N), np.float32)
    oT_sg = np.zeros((RV, N), np.float32)
    for h in range(HPC):
        hs = slice(h * 64, (h + 1) * 64)
        sT = qkT_mu[512 + h * 64:512 + (h + 1) * 64].T @ qkT_mu[hs]  # [kt, qt]
        e = qb(np.exp(SCALE * sT))
        den = e.sum(0, keepdims=True)
        db = qb(1.0 / den)
        p = qb(e * db)
        oT_mu[hs] = (v_mu[:, hs].T @ e) * db
        sdT = qkT_sg[512 + h * 64:512 + (h + 1) * 64].T @ qkT_sg[hs]
        t = qb((p - 1.0) * p)
        w = qb(qb(t * t) * sdT)
        oT_sg[hs] = (v_sg[:, hs].T @ w) * ISWB
    oT_mu = qb(oT_mu)
    oT_sg8 = q8(oT_sg - CBIAS)
    a2o = q8(oT_sg8 + qb(oT_mu * oT_mu))
    wsig_o = q8(np.exp(m["wo_sr"]))
    wmu2_o = q8(m["wo_mu"] ** 2)
    comp = CBIAS * (wsig_o + wmu2_o).sum(0, keepdims=True).T  # [D,1]
    yT_mu = (m["wo_mu"].T @ oT_mu) * ISQ512
    yT_sg = (wsig_o.T @ a2o + wmu2_o.T @ oT_sg8 + comp) * IS512
    return yT_mu.astype(np.float32), yT_sg.astype(np.float32)


def kernel(mu, sigma, ln_gamma, ln_beta, Wqkv_mu, Wqkv_sigma_raw, Wout_mu,
           Wout_sigma_raw, _trace=False):
    from concourse.bass_utils import run_bass_kernel_spmd

    nc = _get_nc()
    args = (mu, sigma, ln_gamma, ln_beta, Wqkv_mu, Wqkv_sigma_raw, Wout_mu,
            Wout_sigma_raw)
    in_maps = [_prep_core_inputs(c, *args) for c in range(8)]
    res = run_bass_kernel_spmd(nc, in_maps, list(range(8)), trace=_trace)
    out_mu = np.zeros((B, N, D), np.float32)
    out_sg = np.zeros((B, N, D), np.float32)
    for c in range(8):
        b = c // 2
        out_mu[b] += res.results[c]["yT_mu"].T
        out_sg[b] += res.results[c]["yT_sg"].T
    if _trace:
        kernel._last_result = res
    return out_mu, out_sg
